# revision 10
# baseline (speedup 1.0000x reference)
"""Causal GQA multi-head attention on 8 TRN2 NeuronCores.

Sharding: data-parallel over batch (B=8 -> one batch element per core,
weights replicated, no collectives).

Per-core kernel (T=1024, C=576, 9 q-heads / 3 kv-heads, hd=64):
  - x [T, C] f32 is loaded and transposed on-chip (PE transpose) into
    xT [C(+1 ones row), T] bf16.
  - qT = (Wq|bq)^T-style projections: the bias is folded into the matmul
    by appending a ones row to xT and a bias row to each weight matrix.
    qT/kT are produced channel-major (what attention needs); v is produced
    token-major with a ones column appended (denominator trick).
  - scores are computed transposed, S^T[tk, tq] = (k_tile)^T-block @ qT,
    exp() on ScalarE (scale 1/sqrt(hd) folded into q), causal handled by
    only computing lower blocks + a 0/1 upper-triangular mask multiply on
    the diagonal blocks.
  - y[tq, d] (+ row-sum column l[tq]) accumulates P~^T-block.T @ [v|1] in
    PSUM; normalize with reciprocal + per-partition tensor_scalar.
  - y is PE-transposed to yT (+ones row) and projected by (Wo|bo).
"""

import sys

for _p in ("/opt/trn_rl_repo",):
    if _p not in sys.path:
        sys.path.insert(0, _p)

from contextlib import ExitStack

import ml_dtypes
import numpy as np

import concourse.bass as bass
import concourse.mybir as mybir
import concourse.tile as tile
from concourse import bacc
from concourse.bass_utils import run_bass_kernel_spmd
from concourse.masks import make_identity, make_upper_triangular

B, T, C = 8, 1024, 576
NH, NKV, HD = 9, 3, 64
KVC = C // NKV * NKV // 3  # 192
KVC = 192
NREP = NH // NKV  # 3
NKT = 5  # channel k-tiles: 4 x 128 + 64(+1 ones row)
NTT = T // 128  # 8 token tiles
F32 = mybir.dt.float32
BF16 = mybir.dt.bfloat16
SCALE = 1.0 / float(np.sqrt(HD))

N_CORES = 8


def _cw(ki):
    """channel-tile row count (without ones row)"""
    return 128 if ki < NKT - 1 else C - 128 * (NKT - 1)  # 64


def _kw(ki):
    """channel-tile row count as matmul K (incl. ones row on last tile)"""
    return 128 if ki < NKT - 1 else C - 128 * (NKT - 1) + 1  # 65


def build_kernel(tc, ctx, x, wq, wk, wv, wo, out):
    nc = tc.nc

    consts = ctx.enter_context(tc.tile_pool(name="consts", bufs=1))
    persist = ctx.enter_context(tc.tile_pool(name="persist", bufs=1))

    # --- constants: weights, identities, diag mask -------------------------
    ident_f32 = consts.tile([128, 128], F32, tag="idf")
    make_identity(nc, ident_f32)
    ident_bf16 = consts.tile([128, 128], BF16, tag="idb")
    make_identity(nc, ident_bf16)
    # 1 on/above diagonal, 0 below: multiplies exp(S^T) diagonal blocks
    # (keep tk <= tq).
    m01 = consts.tile([128, 128], BF16, tag="m01")
    make_upper_triangular(nc, m01, val=1.0, diag=True)

    wq_sb, wk_sb, wv_sb, wo_sb = [], [], [], []
    for ki in range(NKT):
        kw = _kw(ki)
        r0 = 128 * ki
        wq_t = consts.tile([kw, C], BF16, tag=f"wq{ki}")
        nc.sync.dma_start(out=wq_t, in_=wq[r0 : r0 + kw, :])
        wq_sb.append(wq_t)
        wk_t = consts.tile([kw, KVC], BF16, tag=f"wk{ki}")
        nc.sync.dma_start(out=wk_t, in_=wk[r0 : r0 + kw, :])
        wk_sb.append(wk_t)
        wv_t = consts.tile([kw, KVC], BF16, tag=f"wv{ki}")
        nc.sync.dma_start(out=wv_t, in_=wv[r0 : r0 + kw, :])
        wv_sb.append(wv_t)
        wo_t = consts.tile([kw, C], BF16, tag=f"wo{ki}")
        nc.sync.dma_start(out=wo_t, in_=wo[r0 : r0 + kw, :])
        wo_sb.append(wo_t)

    # --- persistent activations -------------------------------------------
    xT = []  # channel-major x, last tile has ones row at row 64
    yT = []  # channel-major attention out, ones row likewise
    for ki in range(NKT):
        kw = _kw(ki)
        xT_t = persist.tile([kw, T], BF16, tag=f"xT{ki}")
        xT.append(xT_t)
        yT_t = persist.tile([kw, T], BF16, tag=f"yT{ki}")
        yT.append(yT_t)
    nc.vector.memset(xT[NKT - 1][_cw(NKT - 1) : _kw(NKT - 1), :], 1.0)
    nc.vector.memset(yT[NKT - 1][_cw(NKT - 1) : _kw(NKT - 1), :], 1.0)

    # per-head channel-major q (scaled by 1/sqrt(hd)) and per-kv-head k,
    # each at base partition 0 (matmul requires lhsT/rhs base match)
    qT = []
    for h in range(NH):
        qT_t = persist.tile([HD, T], BF16, tag=f"qT{h}")
        qT.append(qT_t)
    kT = []
    for g in range(NKV):
        kT_t = persist.tile([HD, T], BF16, tag=f"kT{g}")
        kT.append(kT_t)

    v_aug = []  # per token tile: [128, NKV, 65]; col 64 = ones
    for tt in range(NTT):
        v_t = persist.tile([128, NKV, HD + 1], BF16, tag=f"v{tt}")
        nc.vector.memset(v_t[:, :, HD : HD + 1], 1.0)
        v_aug.append(v_t)

    # softmax denominators (row 64 of each head's y^T psum), gathered by DMA
    l9 = persist.tile([NH, T], BF16, tag="l9")
    # per-token reciprocal denominators broadcast to channel rows (matches yT
    # tiling; row 64 of Z[4] is 1.0 to leave the ones row alone)
    Z = []
    for ki in range(NKT):
        Z_t = persist.tile([_kw(ki), T], BF16, tag=f"Z{ki}")
        Z.append(Z_t)
    nc.vector.memset(Z[NKT - 1][_cw(NKT - 1) : _kw(NKT - 1), :], 1.0)

    # --- phase 1: load x, transpose to xT ---------------------------------
    with (
        tc.tile_pool(name="xload", bufs=3) as xload,
        tc.tile_pool(name="tps", bufs=4, space="PSUM") as tps,
    ):
        for tt in range(NTT):
            xt = xload.tile([128, C], F32, tag="xt")
            nc.sync.dma_start(out=xt, in_=x[128 * tt : 128 * (tt + 1), :])
            for ki in range(NKT):
                cw = _cw(ki)
                ps = tps.tile([128, 128], F32, tag="tp")
                nc.tensor.transpose(
                    ps[:cw, :], xt[:, 128 * ki : 128 * ki + cw], ident_f32
                )
                nc.vector.tensor_copy(
                    xT[ki][0:cw, 128 * tt : 128 * (tt + 1)], ps[:cw, :]
                )

    # --- phase 2: q/k/v projections ---------------------------------------
    with tc.tile_pool(name="qkv_ps", bufs=2, space="PSUM") as qkv_ps:
        # qT[c_out, t] = sum_c wq[c, c_out] * xT[c, t]  (+bias via ones row)
        for h in range(NH):
            for ni in range(2):
                n0 = 512 * ni
                ps = qkv_ps.tile([HD, 512], F32, tag="proj")
                for ki in range(NKT):
                    nc.tensor.matmul(
                        ps,
                        lhsT=wq_sb[ki][:, HD * h : HD * (h + 1)],
                        rhs=xT[ki][:, n0 : n0 + 512],
                        start=(ki == 0),
                        stop=(ki == NKT - 1),
                    )
                # scale by 1/sqrt(hd) while evacuating (cast to bf16)
                nc.vector.tensor_scalar_mul(qT[h][:, n0 : n0 + 512], ps, SCALE)
        # kT[c_out, t]
        for g in range(NKV):
            for ni in range(2):
                n0 = 512 * ni
                ps = qkv_ps.tile([HD, 512], F32, tag="proj")
                for ki in range(NKT):
                    nc.tensor.matmul(
                        ps,
                        lhsT=wk_sb[ki][:, HD * g : HD * (g + 1)],
                        rhs=xT[ki][:, n0 : n0 + 512],
                        start=(ki == 0),
                        stop=(ki == NKT - 1),
                    )
                nc.vector.tensor_copy(kT[g][:, n0 : n0 + 512], ps)
        # v[t, c'] token-major
        for tt in range(NTT):
            ps = qkv_ps.tile([128, KVC], F32, tag="vproj")
            for ki in range(NKT):
                nc.tensor.matmul(
                    ps,
                    lhsT=xT[ki][:, 128 * tt : 128 * (tt + 1)],
                    rhs=wv_sb[ki],
                    start=(ki == 0),
                    stop=(ki == NKT - 1),
                )
            nc.vector.tensor_copy(
                v_aug[tt][:, :, 0:HD],
                ps.rearrange("p (g d) -> p g d", g=NKV),
            )

    # --- phase 3: attention, one kv-group (3 q-heads) at a time -----------
    # Causal j-blocks packed into 4 psum/exp tiles per head with no junk:
    #   tile 0: j0 @ 0 (1024) | j4 @ 1024 (512)   -> exp N=1536
    #   tile 1: j1 @ 0 (896)  | j7 @ 896  (128)   -> exp N=1024
    #   tile 2: j2 @ 0 (768)  | j6 @ 768  (256)   -> exp N=1024
    #   tile 3: j3 @ 0 (640)  | j5 @ 640  (384)   -> exp N=1024
    J_MAP = {0: (0, 0), 4: (0, 1024), 1: (1, 0), 7: (1, 896),
             2: (2, 0), 6: (2, 768), 3: (3, 0), 5: (3, 640)}
    EXP_LEN = {0: 1536, 1: 1024, 2: 1024, 3: 1024}
    for g in range(NKV):
        with (
            tc.tile_pool(name=f"pexp{g}", bufs=1) as pexp,
            tc.tile_pool(name=f"sc{g}", bufs=2, space="PSUM") as sc_ps,
            tc.tile_pool(name=f"pv{g}", bufs=1, space="PSUM") as pv_ps,
            tc.tile_pool(name=f"ybf{g}", bufs=3) as ybf_pool,
        ):
            for hl in range(NREP):
                h = NREP * g + hl
                ptiles = []
                for t in range(4):
                    ps = sc_ps.tile([128, 1536], F32, tag="s")
                    for j, (tj, off) in J_MAP.items():
                        if tj != t:
                            continue
                        nq = T - 128 * j
                        c = off  # chunk at 512 (psum bank) boundaries
                        while c < off + nq:
                            ce = min((c // 512 + 1) * 512, off + nq)
                            nc.tensor.matmul(
                                ps[:, c:ce],
                                lhsT=kT[g][:, 128 * j : 128 * (j + 1)],
                                rhs=qT[h][
                                    :, 128 * j + (c - off) : 128 * j + (ce - off)
                                ],
                                start=True,
                                stop=True,
                            )
                            c = ce
                    pt = pexp.tile([128, 1536], BF16, tag=f"p{hl}_{t}")
                    nL = EXP_LEN[t]
                    nc.scalar.activation(
                        pt[:, 0:nL], ps[:, 0:nL], mybir.ActivationFunctionType.Exp
                    )
                    # causal mask inside each diagonal block: zero tk > tq
                    for j, (tj, off) in J_MAP.items():
                        if tj == t:
                            nc.vector.tensor_mul(
                                pt[:, off : off + 128], pt[:, off : off + 128], m01
                            )
                    ptiles.append(pt)

                # yT_aug[h] = [v|1].T-accum: psum rows 0..63 = y^T, row 64 = l
                yps = pv_ps.tile([HD + 1, T], F32, tag="yh")
                for j in range(NTT):
                    t, off = J_MAP[j]
                    if j < 4:
                        chunks = [(128 * j, 512 - 128 * j), (512, 512)]
                    else:
                        chunks = [(128 * j, 1024 - 128 * j)]
                    for c0, cn in chunks:
                        nc.tensor.matmul(
                            yps[:, c0 : c0 + cn],
                            lhsT=v_aug[j][:, g, :],
                            rhs=ptiles[t][:, off + c0 - 128 * j : off + c0 - 128 * j + cn],
                            start=(j == 0),
                            stop=((j == 3 and c0 < 512) or j == 7),
                        )
                ybf = ybf_pool.tile([HD + 1, T], BF16, tag="ybf")
                nc.vector.tensor_copy(ybf, yps)
                ki, po = divmod(HD * h, 128)
                # route channel rows into yT (partition shift via DMA), and
                # the denominator row into l9
                nc.sync.dma_start(out=yT[ki][po : po + HD, :], in_=ybf[0:HD, :])
                nc.sync.dma_start(out=l9[h : h + 1, :], in_=ybf[HD : HD + 1, :])

    # --- phase 4: normalize yT by broadcast reciprocal denominators -------
    with (
        tc.tile_pool(name="zp", bufs=1) as zpool,
        tc.tile_pool(name="zdram", bufs=1, space="DRAM") as zdram_pool,
    ):
        l9f = zpool.tile([NH, T], F32, tag="l9f")
        nc.vector.tensor_copy(l9f, l9)
        zf = zpool.tile([NH, T], F32, tag="zf")
        nc.vector.reciprocal_approx_fast(zf, l9f)
        z9b = zpool.tile([NH, T], BF16, tag="z9b")
        nc.vector.tensor_copy(z9b, zf)
        zdram = zdram_pool.tile([NH, T], BF16)
        nc.sync.dma_start(out=zdram, in_=z9b)
        for h in range(NH):
            ki, po = divmod(HD * h, 128)
            zsl = zdram[h : h + 1, :]
            zbcast = bass.AP(
                tensor=zsl.tensor,
                offset=zsl.offset,
                ap=[[0, HD], list(zsl.ap[1])],
            )
            nc.gpsimd.dma_start(out=Z[ki][po : po + HD, :], in_=zbcast)
        for ki in range(NKT):
            rows = 2 * HD if ki < NKT - 1 else HD
            nc.vector.tensor_mul(
                yT[ki][0:rows, :], yT[ki][0:rows, :], Z[ki][0:rows, :]
            )

    # --- phase 5: output projection ---------------------------------------
    with (
        tc.tile_pool(name="ops", bufs=2, space="PSUM") as ops,
        tc.tile_pool(name="osb", bufs=3) as osb,
    ):
        for tt in range(NTT):
            ps = ops.tile([128, 1024], F32, tag="o")
            for c0, cn in ((0, 512), (512, 64)):
                for ki in range(NKT):
                    nc.tensor.matmul(
                        ps[:, c0 : c0 + cn],
                        lhsT=yT[ki][:, 128 * tt : 128 * (tt + 1)],
                        rhs=wo_sb[ki][:, c0 : c0 + cn],
                        start=(ki == 0),
                        stop=(ki == NKT - 1),
                    )
            o_sb = osb.tile([128, C], F32, tag="ot")
            nc.vector.tensor_copy(o_sb, ps[:, 0:C])
            nc.sync.dma_start(out=out[128 * tt : 128 * (tt + 1), :], in_=o_sb)


def build_bass():
    # Bacc (not raw Bass): its finalize() runs move_matmul_waits_to_ldweights
    # + generate_event_semaphores, required to satisfy the 1-wait-per-
    # instruction hardware constraint that walrus enforces.
    nc = bacc.Bacc("TRN2", target_bir_lowering=False)
    x = nc.declare_dram_parameter("x", [T, C], F32, isOutput=False)
    wq = nc.declare_dram_parameter("wq", [C + 1, C], BF16, isOutput=False)
    wk = nc.declare_dram_parameter("wk", [C + 1, KVC], BF16, isOutput=False)
    wv = nc.declare_dram_parameter("wv", [C + 1, KVC], BF16, isOutput=False)
    wo = nc.declare_dram_parameter("wo", [C + 1, C], BF16, isOutput=False)
    out = nc.declare_dram_parameter("out", [T, C], F32, isOutput=True)
    with tile.TileContext(nc) as tc, ExitStack() as ctx:
        build_kernel(tc, ctx, x[:], wq[:], wk[:], wv[:], wo[:], out[:])
    nc.finalize()  # runs Bacc.compile(): reg alloc + wait splitting
    return nc


_NC_CACHE = None


def _get_nc():
    global _NC_CACHE
    if _NC_CACHE is None:
        _NC_CACHE = build_bass()
    return _NC_CACHE


def prep_inputs(x, Wq, bq, Wk, bk, Wv, bv, Wo, bo):
    """Host-side: fold biases into an extra weight row, cast weights bf16."""
    bf = ml_dtypes.bfloat16
    wq = np.concatenate([Wq, bq[None, :]], axis=0).astype(bf)
    wk = np.concatenate([Wk, bk[None, :]], axis=0).astype(bf)
    wv = np.concatenate([Wv, bv[None, :]], axis=0).astype(bf)
    wo = np.concatenate([Wo, bo[None, :]], axis=0).astype(bf)
    x = np.ascontiguousarray(np.asarray(x, dtype=np.float32))
    in_maps = [
        {"x": x[b], "wq": wq, "wk": wk, "wv": wv, "wo": wo} for b in range(N_CORES)
    ]
    return in_maps


def kernel(x, Wq, bq, Wk, bk, Wv, bv, Wo, bo, _trace=False, _trace_kwargs=None):
    nc = _get_nc()
    in_maps = prep_inputs(x, Wq, bq, Wk, bk, Wv, bv, Wo, bo)
    res = run_bass_kernel_spmd(
        nc,
        in_maps,
        core_ids=list(range(N_CORES)),
        trace=_trace,
        **(_trace_kwargs or {}),
    )
    out = np.stack([res.results[b]["out"] for b in range(N_CORES)], axis=0)
    if _trace:
        return out.astype(np.float32), res
    return out.astype(np.float32)


# revision 15
# speedup vs baseline: 1.1212x; 1.1212x over previous
"""Causal GQA multi-head attention on 8 TRN2 NeuronCores.

Sharding: data-parallel over batch (B=8 -> one batch element per core,
weights replicated, no collectives).

Per-core kernel (T=1024, C=576, 9 q-heads / 3 kv-heads, hd=64):
  - x arrives host-padded to [T, 640] bf16 (col 576 = 1.0 ones column for
    the bias trick, 577.. = 0) and is transposed on-chip by the DMA xbar
    into xT [128*5, T] channel-major tiles.
  - Projections fold biases into the matmul via the ones row; qT/kT are
    produced channel-major per head (scores need them that way), v
    token-major with a ones column (softmax denominator trick).
  - Scores are computed transposed, S^T[tk, tq] = kT-block.T-free @ qT,
    causal handled by only computing lower blocks; the 8 j-blocks of a
    head are packed into 4 psum tiles so exp() is 4 ScalarE ops per head
    with zero junk columns. Diagonal-block masking = multiply by a 0/1
    upper-triangular bf16 mask on GpSimd (post-exp).
  - PV accumulates yT_aug[h] = [v|1].T @ P~^T directly in transposed
    layout ([65, T] psum; row 64 = softmax denominator l), so no
    transposes of P~ or y are ever needed. DMA routes the result rows
    into the shared yT tiles; l rows are gathered, inverted with
    reciprocal_approx_fast, broadcast back over 64 partitions via DMA,
    and multiplied in.
  - QKV projection matmuls are emitted inside the attention group loop
    (sharing its psum slots) so the TensorE always has work while
    ScalarE grinds through exp() -- keeps the PE HAM clock warm.
  - out = yT.T @ (Wo|bo) with the ones row supplying the bias.
"""

import sys

for _p in ("/opt/trn_rl_repo",):
    if _p not in sys.path:
        sys.path.insert(0, _p)

from contextlib import ExitStack

import ml_dtypes
import numpy as np

import concourse.bass as bass
import concourse.mybir as mybir
import concourse.tile as tile
from concourse import bacc
from concourse.bass_utils import run_bass_kernel_spmd
from concourse.masks import make_upper_triangular

B, T, C = 8, 1024, 576
NH, NKV, HD = 9, 3, 64
KVC = 192
NREP = NH // NKV  # 3
NKT = 5  # channel k-tiles: 4 x 128 + 64(+1 ones row)
NTT = T // 128  # 8 token tiles
CPAD = 640  # host-padded x width: C + ones col + zeros
F32 = mybir.dt.float32
BF16 = mybir.dt.bfloat16
SCALE = 1.0 / float(np.sqrt(HD))

N_CORES = 8

# causal j-block -> (exp tile, column offset); packs the 8 blocks of a head
# into 4 psum/exp tiles with no junk columns:
#   tile 0: j0 @ 0 (1024) | j4 @ 1024 (512)   -> exp N=1536
#   tile 1: j1 @ 0 (896)  | j7 @ 896  (128)   -> exp N=1024
#   tile 2: j2 @ 0 (768)  | j6 @ 768  (256)   -> exp N=1024
#   tile 3: j3 @ 0 (640)  | j5 @ 640  (384)   -> exp N=1024
J_MAP = {0: (0, 0), 4: (0, 1024), 1: (1, 0), 7: (1, 896),
         2: (2, 0), 6: (2, 768), 3: (3, 0), 5: (3, 640)}
EXP_LEN = {0: 1536, 1: 1024, 2: 1024, 3: 1024}


def _cw(ki):
    """channel-tile row count (without ones row)"""
    return 128 if ki < NKT - 1 else C - 128 * (NKT - 1)  # 64


def _kw(ki):
    """channel-tile row count as matmul K (incl. ones row on last tile)"""
    return 128 if ki < NKT - 1 else C - 128 * (NKT - 1) + 1  # 65


def build_kernel(tc, ctx, x, wq, wk, wv, wo, out):
    nc = tc.nc

    consts = ctx.enter_context(tc.tile_pool(name="consts", bufs=1))
    persist = ctx.enter_context(tc.tile_pool(name="persist", bufs=1))
    dram_pool = ctx.enter_context(tc.tile_pool(name="dram", bufs=1, space="DRAM"))

    # 1 on/above diagonal, 0 below: multiplies exp(S^T) diagonal blocks
    # (keep tk <= tq).
    m01 = consts.tile([128, 128], BF16, tag="m01")
    make_upper_triangular(nc, m01, val=1.0, diag=True)

    wq_sb, wk_sb, wv_sb, wo_sb = [], [], [], []
    for ki in range(NKT):
        kw = _kw(ki)
        r0 = 128 * ki
        wq_t = consts.tile([kw, C], BF16, tag=f"wq{ki}")
        nc.sync.dma_start(out=wq_t, in_=wq[r0 : r0 + kw, :])
        wq_sb.append(wq_t)
        wk_t = consts.tile([kw, KVC], BF16, tag=f"wk{ki}")
        nc.sync.dma_start(out=wk_t, in_=wk[r0 : r0 + kw, :])
        wk_sb.append(wk_t)
        wv_t = consts.tile([kw, KVC], BF16, tag=f"wv{ki}")
        nc.sync.dma_start(out=wv_t, in_=wv[r0 : r0 + kw, :])
        wv_sb.append(wv_t)
        wo_t = consts.tile([kw, C], BF16, tag=f"wo{ki}")
        nc.sync.dma_start(out=wo_t, in_=wo[r0 : r0 + kw, :])
        wo_sb.append(wo_t)

    # --- persistent activations -------------------------------------------
    # xT via DMA xbar transpose straight from (host-padded, bf16) x.
    # Tile 4 rows: 64 real channels + ones row (x col 576) + zero junk.
    xT = []
    for ki in range(NKT):
        xT_t = persist.tile([128, T], BF16, tag=f"xT{ki}")
        nc.sync.dma_start_transpose(xT_t, x[:, 128 * ki : 128 * (ki + 1)])
        xT.append(xT_t)

    yT = []  # channel-major attention out, ones row on last tile
    for ki in range(NKT):
        yT_t = persist.tile([_kw(ki), T], BF16, tag=f"yT{ki}")
        yT.append(yT_t)
    nc.vector.memset(yT[NKT - 1][_cw(NKT - 1) : _kw(NKT - 1), :], 1.0)

    # per-head channel-major q (scaled by 1/sqrt(hd)) and per-kv-head k,
    # each at base partition 0 (matmul requires lhsT/rhs base match)
    qT = [persist.tile([HD, T], BF16, tag=f"qT{h}", name=f"qT{h}") for h in range(NH)]
    kT = [persist.tile([HD, T], BF16, tag=f"kT{g}", name=f"kT{g}") for g in range(NKV)]

    v_aug = []  # per token tile: [128, NKV, 65]; col 64 = ones
    for tt in range(NTT):
        v_t = persist.tile([128, NKV, HD + 1], BF16, tag=f"v{tt}", name=f"v{tt}")
        nc.vector.memset(v_t[:, :, HD : HD + 1], 1.0)
        v_aug.append(v_t)

    # reciprocal denominators broadcast to 64 rows per head (yT layout)
    Z = [persist.tile([_cw(ki), T], BF16, tag=f"Z{ki}", name=f"Z{ki}")
         for ki in range(NKT)]
    zdram = dram_pool.tile([NH, T], BF16)

    # --- attention (with q/k/v projections interleaved per kv-group) ------
    for g in range(NKV):
        with (
            tc.tile_pool(name=f"pexp{g}", bufs=1) as pexp,
            tc.tile_pool(name=f"sc{g}", bufs=2, space="PSUM") as sc_ps,
            tc.tile_pool(name=f"pv{g}", bufs=1, space="PSUM") as pv_ps,
            tc.tile_pool(name=f"ybf{g}", bufs=3) as ybf_pool,
            tc.tile_pool(name=f"zp{g}", bufs=1) as zpool,
        ):
            # this group's softmax denominator rows (DMA-gathered, base 0)
            l3 = zpool.tile([NREP, T], BF16, tag="l3")
            # v projection (token-major, all kv heads) -- group 0 only
            if g == 0:
                for tt in range(NTT):
                    ps = sc_ps.tile([128, 1536], F32, tag="s", name=f"vp{tt}")
                    for ki in range(NKT):
                        nc.tensor.matmul(
                            ps[:, 0:KVC],
                            lhsT=xT[ki][: _kw(ki), 128 * tt : 128 * (tt + 1)],
                            rhs=wv_sb[ki],
                            start=(ki == 0),
                            stop=(ki == NKT - 1),
                        )
                    nc.vector.tensor_copy(
                        v_aug[tt][:, :, 0:HD],
                        ps[:, 0:KVC].rearrange("p (a d) -> p a d", a=NKV),
                    )
            # kT for this group
            for ni in range(2):
                n0 = 512 * ni
                ps = sc_ps.tile([128, 1536], F32, tag="s", name=f"kp{g}_{ni}")
                for ki in range(NKT):
                    nc.tensor.matmul(
                        ps[:HD, 0:512],
                        lhsT=wk_sb[ki][:, HD * g : HD * (g + 1)],
                        rhs=xT[ki][: _kw(ki), n0 : n0 + 512],
                        start=(ki == 0),
                        stop=(ki == NKT - 1),
                    )
                nc.vector.tensor_copy(kT[g][:, n0 : n0 + 512], ps[:HD, 0:512])
            # qT for this group's heads (scaled while evacuating)
            for hl in range(NREP):
                h = NREP * g + hl
                for ni in range(2):
                    n0 = 512 * ni
                    ps = sc_ps.tile([128, 1536], F32, tag="s", name=f"qp{h}_{ni}")
                    for ki in range(NKT):
                        nc.tensor.matmul(
                            ps[:HD, 0:512],
                            lhsT=wq_sb[ki][:, HD * h : HD * (h + 1)],
                            rhs=xT[ki][: _kw(ki), n0 : n0 + 512],
                            start=(ki == 0),
                            stop=(ki == NKT - 1),
                        )
                    nc.vector.tensor_scalar_mul(
                        qT[h][:, n0 : n0 + 512], ps[:HD, 0:512], SCALE
                    )

            for hl in range(NREP):
                h = NREP * g + hl
                ptiles = []
                for t in range(4):
                    ps = sc_ps.tile([128, 1536], F32, tag="s", name=f"s{h}_{t}")
                    for j, (tj, off) in J_MAP.items():
                        if tj != t:
                            continue
                        nq = T - 128 * j
                        c = off  # chunk at 512 (psum bank) boundaries
                        while c < off + nq:
                            ce = min((c // 512 + 1) * 512, off + nq)
                            nc.tensor.matmul(
                                ps[:, c:ce],
                                lhsT=kT[g][:, 128 * j : 128 * (j + 1)],
                                rhs=qT[h][
                                    :, 128 * j + (c - off) : 128 * j + (ce - off)
                                ],
                                start=True,
                                stop=True,
                            )
                            c = ce
                    pt = pexp.tile([128, 1536], BF16, tag=f"p{hl}_{t}",
                                   name=f"p{hl}_{t}")
                    nL = EXP_LEN[t]
                    nc.scalar.activation(
                        pt[:, 0:nL], ps[:, 0:nL], mybir.ActivationFunctionType.Exp
                    )
                    # causal mask inside each diagonal block: zero tk > tq
                    for j, (tj, off) in J_MAP.items():
                        if tj == t:
                            nc.gpsimd.tensor_mul(
                                pt[:, off : off + 128], pt[:, off : off + 128], m01
                            )
                    ptiles.append(pt)

                # yT_aug[h]: psum rows 0..63 = y^T, row 64 = denominator l
                yps = pv_ps.tile([HD + 1, T], F32, tag="yh", name=f"yps{h}")
                for j in range(NTT):
                    t, off = J_MAP[j]
                    if j < 4:
                        chunks = [(128 * j, 512 - 128 * j), (512, 512)]
                    else:
                        chunks = [(128 * j, 1024 - 128 * j)]
                    for c0, cn in chunks:
                        nc.tensor.matmul(
                            yps[:, c0 : c0 + cn],
                            lhsT=v_aug[j][:, g, :],
                            rhs=ptiles[t][
                                :, off + c0 - 128 * j : off + c0 - 128 * j + cn
                            ],
                            start=(j == 0),
                            stop=((j == 3 and c0 < 512) or j == 7),
                        )
                ybf = ybf_pool.tile([HD + 1, T], BF16, tag="ybf", name=f"ybf{h}")
                nc.vector.tensor_copy(ybf, yps)
                ki, po = divmod(HD * h, 128)
                # route channel rows into yT (partition shift via DMA) and
                # the denominator row into l9
                nc.sync.dma_start(out=yT[ki][po : po + HD, :], in_=ybf[0:HD, :])
                nc.sync.dma_start(out=l3[hl : hl + 1, :], in_=ybf[HD : HD + 1, :])

            # per-group normalization: z = 1/l, broadcast, multiply into yT
            l3f = zpool.tile([NREP, T], F32, tag="l3f")
            nc.vector.tensor_copy(l3f, l3)
            z3f = zpool.tile([NREP, T], F32, tag="z3f")
            nc.vector.reciprocal_approx_fast(z3f, l3f)
            z3b = zpool.tile([NREP, T], BF16, tag="z3b")
            nc.vector.tensor_copy(z3b, z3f)
            nc.sync.dma_start(out=zdram[NREP * g : NREP * (g + 1), :], in_=z3b)
            for hl in range(NREP):
                h = NREP * g + hl
                ki, po = divmod(HD * h, 128)
                zsl = zdram[h : h + 1, :]
                zbcast = bass.AP(
                    tensor=zsl.tensor,
                    offset=zsl.offset,
                    ap=[[0, HD], list(zsl.ap[1])],
                )
                nc.gpsimd.dma_start(out=Z[ki][po : po + HD, :], in_=zbcast)
                nc.vector.tensor_mul(
                    yT[ki][po : po + HD, :],
                    yT[ki][po : po + HD, :],
                    Z[ki][po : po + HD, :],
                )

    # --- output projection -------------------------------------------------
    with (
        tc.tile_pool(name="ops", bufs=2, space="PSUM") as ops,
        tc.tile_pool(name="osb", bufs=3) as osb,
    ):
        for tt in range(NTT):
            ps = ops.tile([128, 1024], F32, tag="o")
            for c0, cn in ((0, 512), (512, 64)):
                for ki in range(NKT):
                    nc.tensor.matmul(
                        ps[:, c0 : c0 + cn],
                        lhsT=yT[ki][:, 128 * tt : 128 * (tt + 1)],
                        rhs=wo_sb[ki][:, c0 : c0 + cn],
                        start=(ki == 0),
                        stop=(ki == NKT - 1),
                    )
            o_sb = osb.tile([128, C], F32, tag="ot")
            nc.vector.tensor_copy(o_sb, ps[:, 0:C])
            nc.sync.dma_start(out=out[128 * tt : 128 * (tt + 1), :], in_=o_sb)


def build_bass():
    # Bacc (not raw Bass): its finalize() runs move_matmul_waits_to_ldweights
    # + generate_event_semaphores, required to satisfy the 1-wait-per-
    # instruction hardware constraint that walrus enforces.
    nc = bacc.Bacc("TRN2", target_bir_lowering=False)
    x = nc.declare_dram_parameter("x", [T, CPAD], BF16, isOutput=False)
    wq = nc.declare_dram_parameter("wq", [C + 1, C], BF16, isOutput=False)
    wk = nc.declare_dram_parameter("wk", [C + 1, KVC], BF16, isOutput=False)
    wv = nc.declare_dram_parameter("wv", [C + 1, KVC], BF16, isOutput=False)
    wo = nc.declare_dram_parameter("wo", [C + 1, C], BF16, isOutput=False)
    out = nc.declare_dram_parameter("out", [T, C], F32, isOutput=True)
    with tile.TileContext(nc) as tc, ExitStack() as ctx:
        build_kernel(tc, ctx, x[:], wq[:], wk[:], wv[:], wo[:], out[:])
    nc.finalize()  # runs Bacc.compile(): reg alloc + wait splitting
    return nc


_NC_CACHE = None


def _get_nc():
    global _NC_CACHE
    if _NC_CACHE is None:
        _NC_CACHE = build_bass()
    return _NC_CACHE


def prep_inputs(x, Wq, bq, Wk, bk, Wv, bv, Wo, bo):
    """Host-side: fold biases into an extra weight row, cast to bf16, and
    pad x with a ones column (bias trick) + zeros to the xbar-transpose
    width."""
    bf = ml_dtypes.bfloat16
    wq = np.concatenate([Wq, bq[None, :]], axis=0).astype(bf)
    wk = np.concatenate([Wk, bk[None, :]], axis=0).astype(bf)
    wv = np.concatenate([Wv, bv[None, :]], axis=0).astype(bf)
    wo = np.concatenate([Wo, bo[None, :]], axis=0).astype(bf)
    x = np.asarray(x, dtype=np.float32)
    xp = np.zeros((B, T, CPAD), dtype=bf)
    xp[:, :, :C] = x.astype(bf)
    xp[:, :, C] = 1.0
    in_maps = [
        {"x": np.ascontiguousarray(xp[b]), "wq": wq, "wk": wk, "wv": wv, "wo": wo}
        for b in range(N_CORES)
    ]
    return in_maps


def kernel(x, Wq, bq, Wk, bk, Wv, bv, Wo, bo, _trace=False, _trace_kwargs=None):
    nc = _get_nc()
    in_maps = prep_inputs(x, Wq, bq, Wk, bk, Wv, bv, Wo, bo)
    res = run_bass_kernel_spmd(
        nc,
        in_maps,
        core_ids=list(range(N_CORES)),
        trace=_trace,
        **(_trace_kwargs or {}),
    )
    out = np.stack([res.results[b]["out"] for b in range(N_CORES)], axis=0)
    if _trace:
        return out.astype(np.float32), res
    return out.astype(np.float32)


# revision 28
# speedup vs baseline: 1.1561x; 1.0312x over previous
"""Causal GQA multi-head attention on 8 TRN2 NeuronCores.

Sharding: data-parallel over batch (B=8 -> one batch element per core,
weights replicated, no collectives).

Per-core kernel (T=1024, C=576, 9 q-heads / 3 kv-heads, hd=64):
  - x arrives host-padded to [T, 640] bf16 (col 576 = ones column for the
    bias trick, 577.. = 0) and is transposed on-chip by the DMA xbar into
    channel-major xT tiles. Weights arrive host-padded to 640 rows so each
    loads with a single 3D-strided DMA.
  - Projections fold biases in via the ones row. q is projected in
    head-PAIR m-tiles (M=128) into qP pair tiles; k per kv-head, with
    shifted duplicates so each score pair has its lhsT at both partition
    0 and 64.
  - Scores are computed transposed, S^T[tk, tq] = k-block x qT, causal by
    construction (only lower j-blocks). The two heads of a pair run as
    CONCURRENT matmuls in disjoint PE row-groups (K=64 each). The 8
    j-blocks pack into 4 psum tiles -> 4 exp() ScalarE ops per head with
    zero junk. Diagonal masking = post-exp multiply by a 0/1 triangular
    mask on GpSimd.
  - PV accumulates yT_aug[h] = [v|1].T @ P~^T directly in transposed
    layout ([65, T] psum; row 64 = softmax denominator). DMA routes rows
    into shared yT tiles; denominators are batch-inverted with
    reciprocal_approx_fast and DMA-broadcast, then multiplied in.
  - out = yT.T @ (Wo|bo).
"""

import sys

for _p in ("/opt/trn_rl_repo",):
    if _p not in sys.path:
        sys.path.insert(0, _p)

from contextlib import ExitStack

import ml_dtypes
import numpy as np

import concourse.bass as bass
import concourse.mybir as mybir
import concourse.tile as tile
from concourse import bacc
from concourse.bass_utils import run_bass_kernel_spmd
from concourse.masks import make_upper_triangular

B, T, C = 8, 1024, 576
NH, NKV, HD = 9, 3, 64
KVC = 192
NREP = NH // NKV  # 3
NKT = 5  # channel k-tiles: 4 x 128 + 64(+1 ones row)
NTT = T // 128  # 8 token tiles
CPAD = 640  # host-padded width: C + ones col/row + zeros
F32 = mybir.dt.float32
BF16 = mybir.dt.bfloat16
SCALE = 1.0 / float(np.sqrt(HD))

N_CORES = 8

# causal j-block -> (exp tile, column offset); packs the 8 blocks of a head
# into 4 psum/exp tiles with no junk columns.
J_MAP = {0: (0, 0), 4: (0, 1024), 1: (1, 0), 7: (1, 896),
         2: (2, 0), 6: (2, 768), 3: (3, 0), 5: (3, 640)}
EXP_LEN = {0: 1536, 1: 1024, 2: 1024, 3: 1024}

# head pairs for concurrent score matmuls: (head_even, head_odd)
PAIRS = [(0, 1), (2, 3), (4, 5), (6, 7)]
# Use concurrent PE row-group pairs for scores (False: sequential base-0)
PAIRED_SCORES = False


def _kw(ki):
    """channel-tile rows as matmul K (incl. ones row on last tile)"""
    return 128 if ki < NKT - 1 else C - 128 * (NKT - 1) + 1  # 65


def build_kernel(tc, ctx, x, wq, wk, wv, wo, out):
    nc = tc.nc

    consts = ctx.enter_context(tc.tile_pool(name="consts", bufs=1))
    persist = ctx.enter_context(tc.tile_pool(name="persist", bufs=1))
    dram_pool = ctx.enter_context(tc.tile_pool(name="dram", bufs=1, space="DRAM"))

    # --- xT via DMA xbar transpose, issued first (sync + scalar queues) ---
    xT = []
    for ki in range(NKT):
        xT_t = persist.tile([128, T], BF16, tag=f"xT{ki}", name=f"xT{ki}")
        nc.sync.dma_start_transpose(xT_t, x[:, 128 * ki : 128 * (ki + 1)])
        xT.append(xT_t)

    # --- constants ---------------------------------------------------------
    m01 = consts.tile([128, 128], BF16, tag="m01")
    make_upper_triangular(nc, m01, val=1.0, diag=True)

    # weights: host-interleaved to [128, NKT*cols] so each is ONE contiguous
    # 2D DMA; tile [p, ki, c] = original row 128*ki + p
    def load_w(w_ap, cols, tag):
        t = consts.tile([128, NKT, cols], BF16, tag=tag, name=tag)
        nc.sync.dma_start(out=t, in_=w_ap)
        return t

    wq_sb = load_w(wq, C, "wq")
    wk_sb = load_w(wk, KVC, "wk")
    wv_sb = load_w(wv, KVC, "wv")
    wo_sb = load_w(wo, C, "wo")

    # --- persistent activations -------------------------------------------
    yT = []  # channel-major attention out, ones row on last tile
    for ki in range(NKT):
        yT_t = persist.tile([_kw(ki), T], BF16, tag=f"yT{ki}", name=f"yT{ki}")
        yT.append(yT_t)
    nc.vector.memset(yT[NKT - 1][HD : HD + 1, :], 1.0)

    # q pair tiles (rows 0-63 = even head, 64-127 = odd head), q8 solo
    qP = [persist.tile([128, T], BF16, tag=f"qP{p}", name=f"qP{p}")
          for p in range(4)]
    q8 = persist.tile([HD, T], BF16, tag="q8")
    if not PAIRED_SCORES:  # odd heads shifted to base partition 0
        qOdd = [persist.tile([HD, T], BF16, tag=f"qO{p}", name=f"qO{p}")
                for p in range(4)]
    # k: kpair = [K0|K1] (projection layout), k2 = K2, plus shifted dups so
    # every pair has its lhsT at the right base partition:
    #   dup1 = [K1|K0], dup2 = [.|K2]
    kpair = persist.tile([128, T], BF16, tag="kpair")
    k2 = persist.tile([HD, T], BF16, tag="k2")
    dup1 = persist.tile([128, T], BF16, tag="dup1")
    dup2 = persist.tile([128, T], BF16, tag="dup2")

    v_aug = []  # per token tile: [128, NKV, 65]; col 64 = ones
    for tt in range(NTT):
        v_t = persist.tile([128, NKV, HD + 1], BF16, tag=f"v{tt}", name=f"v{tt}")
        nc.vector.memset(v_t[:, :, HD : HD + 1], 1.0)
        v_aug.append(v_t)

    # softmax denominator rows: heads 0-7 into l8, head 8 into l1
    l8 = persist.tile([8, T], BF16, tag="l8")
    l1 = persist.tile([1, T], BF16, tag="l1")
    # reciprocal denominators broadcast to 64 rows per head (yT layout)
    Z = [persist.tile([128 if ki < NKT - 1 else HD, T], BF16,
                      tag=f"Z{ki}", name=f"Z{ki}") for ki in range(NKT)]
    zdram = dram_pool.tile([NH, T], BF16)

    attn_ctx = ctx.enter_context(ExitStack())
    sc_ps = attn_ctx.enter_context(tc.tile_pool(name="sc", bufs=2, space="PSUM"))
    pv_ps = attn_ctx.enter_context(tc.tile_pool(name="pv", bufs=1, space="PSUM"))
    pexp = attn_ctx.enter_context(tc.tile_pool(name="pexp", bufs=1))
    ybf_pool = attn_ctx.enter_context(tc.tile_pool(name="ybf", bufs=3))
    zpool = attn_ctx.enter_context(tc.tile_pool(name="zp", bufs=1))

    # --- v projection (token-major, ones col added by memset above) -------
    for tt in range(NTT):
        ps = sc_ps.tile([128, 1536], F32, tag="s", name=f"vp{tt}")
        for ki in range(NKT):
            nc.tensor.matmul(
                ps[:, 0:KVC],
                lhsT=xT[ki][: _kw(ki), 128 * tt : 128 * (tt + 1)],
                rhs=wv_sb[: _kw(ki), ki, :],
                start=(ki == 0),
                stop=(ki == NKT - 1),
            )
        nc.vector.tensor_copy(
            v_aug[tt][:, :, 0:HD],
            ps[:, 0:KVC].rearrange("p (a d) -> p a d", a=NKV),
        )

    # --- k projection: [K0|K1] pair m-tile + K2, then shifted dups --------
    for mi, (mw, dst) in enumerate(((128, kpair), (64, k2))):
        for ni in range(2):
            n0 = 512 * ni
            ps = sc_ps.tile([128, 1536], F32, tag="s", name=f"kp{mi}_{ni}")
            for ki in range(NKT):
                nc.tensor.matmul(
                    ps[:mw, 0:512],
                    lhsT=wk_sb[: _kw(ki), ki, 128 * mi : 128 * mi + mw],
                    rhs=xT[ki][: _kw(ki), n0 : n0 + 512],
                    start=(ki == 0),
                    stop=(ki == NKT - 1),
                )
            nc.vector.tensor_copy(dst[:mw, n0 : n0 + 512], ps[:mw, 0:512])
    nc.sync.dma_start(out=dup1[0:HD, :], in_=kpair[HD:128, :])  # K1 -> base 0
    if PAIRED_SCORES:
        nc.sync.dma_start(out=dup1[HD:128, :], in_=kpair[0:HD, :])  # K0 -> b64
        nc.sync.dma_start(out=dup2[HD:128, :], in_=k2[0:HD, :])  # K2 -> b64

    # lhsT source for each pair: (A at base 0, B at base 64 when paired,
    # else base 0)
    K_A = [kpair[0:HD, :], kpair[0:HD, :], dup1[0:HD, :], k2[0:HD, :]]
    if PAIRED_SCORES:
        K_B = [dup1[HD:128, :], kpair[HD:128, :], kpair[HD:128, :],
               dup2[HD:128, :]]
    else:
        K_B = [kpair[0:HD, :], dup1[0:HD, :], dup1[0:HD, :], k2[0:HD, :]]

    # --- attention head pairs ---------------------------------------------
    def emit_scores(ps, lhsT, rhs_src, t):
        """score matmuls for exp-tile t into psum ps (512-col bank chunks)"""
        for j, (tj, off) in J_MAP.items():
            if tj != t:
                continue
            nq = T - 128 * j
            c = off
            while c < off + nq:
                ce = min((c // 512 + 1) * 512, off + nq)
                nc.tensor.matmul(
                    ps[:, c:ce],
                    lhsT=lhsT[:, 128 * j : 128 * (j + 1)],
                    rhs=rhs_src[:, 128 * j + (c - off) : 128 * j + (ce - off)],
                    start=True,
                    stop=True,
                )
                c = ce

    def emit_exp_mask(ps, hl, t, h):
        pt = pexp.tile([128, 1536], BF16, tag=f"p{hl}_{t}", name=f"p{h}_{t}")
        nL = EXP_LEN[t]
        nc.scalar.activation(
            pt[:, 0:nL], ps[:, 0:nL], mybir.ActivationFunctionType.Exp
        )
        for j, (tj, off) in J_MAP.items():
            if tj == t:
                nc.gpsimd.tensor_mul(
                    pt[:, off : off + 128], pt[:, off : off + 128], m01
                )
        return pt

    def emit_pv_evac(h, ptiles, l_dst):
        g = h // NREP
        yps = pv_ps.tile([HD + 1, T], F32, tag="yh", name=f"yps{h}")
        for j in range(NTT):
            t, off = J_MAP[j]
            if j < 4:
                chunks = [(128 * j, 512 - 128 * j), (512, 512)]
            else:
                chunks = [(128 * j, 1024 - 128 * j)]
            for c0, cn in chunks:
                nc.tensor.matmul(
                    yps[:, c0 : c0 + cn],
                    lhsT=v_aug[j][:, g, :],
                    rhs=ptiles[t][:, off + c0 - 128 * j : off + c0 - 128 * j + cn],
                    start=(j == 0),
                    stop=((j == 3 and c0 < 512) or j == 7),
                )
        ybf = ybf_pool.tile([HD + 1, T], BF16, tag="ybf", name=f"ybf{h}")
        nc.vector.tensor_copy(ybf, yps)
        ki, po = divmod(HD * h, 128)
        nc.sync.dma_start(out=yT[ki][po : po + HD, :], in_=ybf[0:HD, :])
        nc.sync.dma_start(out=l_dst, in_=ybf[HD : HD + 1, :])

    for p, (ha, hb) in enumerate(PAIRS):
        # q projection for the pair (M=128 m-tile), scaled while evacuating
        for ni in range(2):
            n0 = 512 * ni
            ps = sc_ps.tile([128, 1536], F32, tag="s", name=f"qp{p}_{ni}")
            for ki in range(NKT):
                nc.tensor.matmul(
                    ps[:, 0:512],
                    lhsT=wq_sb[: _kw(ki), ki, 128 * p : 128 * (p + 1)],
                    rhs=xT[ki][: _kw(ki), n0 : n0 + 512],
                    start=(ki == 0),
                    stop=(ki == NKT - 1),
                )
            nc.vector.tensor_scalar_mul(qP[p][:, n0 : n0 + 512], ps[:, 0:512], SCALE)
        if PAIRED_SCORES:
            rhs_b = qP[p][HD:128, :]
        else:
            nc.sync.dma_start(out=qOdd[p], in_=qP[p][HD:128, :])
            rhs_b = qOdd[p]
        # scores for both heads (concurrent PE row groups when paired)
        pa, pb = [], []
        for t in range(4):
            psA = sc_ps.tile([128, 1536], F32, tag="s", name=f"sA{p}_{t}")
            psB = sc_ps.tile([128, 1536], F32, tag="s", name=f"sB{p}_{t}")
            emit_scores(psA, K_A[p], qP[p][0:HD, :], t)
            emit_scores(psB, K_B[p], rhs_b, t)
            pa.append(emit_exp_mask(psA, 0, t, ha))
            pb.append(emit_exp_mask(psB, 1, t, hb))
        emit_pv_evac(ha, pa, l8[ha : ha + 1, :])
        emit_pv_evac(hb, pb, l8[hb : hb + 1, :])

    # normalize heads 0-7 (overlaps head 8's attention below)
    def z_chain(l_src, rows, zrow0):
        lf = zpool.tile([rows, T], F32, tag=f"lf{zrow0}", name=f"lf{zrow0}")
        nc.vector.tensor_copy(lf, l_src)
        zf = zpool.tile([rows, T], F32, tag=f"zf{zrow0}", name=f"zf{zrow0}")
        nc.vector.reciprocal_approx_fast(zf, lf)
        zb = zpool.tile([rows, T], BF16, tag=f"zb{zrow0}", name=f"zb{zrow0}")
        nc.vector.tensor_copy(zb, zf)
        nc.sync.dma_start(out=zdram[zrow0 : zrow0 + rows, :], in_=zb)
        for h in range(zrow0, zrow0 + rows):
            ki, po = divmod(HD * h, 128)
            zsl = zdram[h : h + 1, :]
            zbcast = bass.AP(
                tensor=zsl.tensor,
                offset=zsl.offset,
                ap=[[0, HD], list(zsl.ap[1])],
            )
            nc.gpsimd.dma_start(out=Z[ki][po : po + HD, :], in_=zbcast)
            nc.vector.tensor_mul(
                yT[ki][po : po + HD, :],
                yT[ki][po : po + HD, :],
                Z[ki][po : po + HD, :],
            )

    z_chain(l8, 8, 0)

    # head 8 (solo, K=64 at base 0 only)
    for ni in range(2):
        n0 = 512 * ni
        ps = sc_ps.tile([128, 1536], F32, tag="s", name=f"qp8_{ni}")
        for ki in range(NKT):
            nc.tensor.matmul(
                ps[:HD, 0:512],
                lhsT=wq_sb[: _kw(ki), ki, 512 : 512 + HD],
                rhs=xT[ki][: _kw(ki), n0 : n0 + 512],
                start=(ki == 0),
                stop=(ki == NKT - 1),
            )
        nc.vector.tensor_scalar_mul(q8[:, n0 : n0 + 512], ps[:HD, 0:512], SCALE)
    p8 = []
    for t in range(4):
        ps = sc_ps.tile([128, 1536], F32, tag="s", name=f"s8_{t}")
        emit_scores(ps, k2[0:HD, :], q8, t)
        p8.append(emit_exp_mask(ps, 0, t, 8))
    emit_pv_evac(8, p8, l1[0:1, :])
    z_chain(l1, 1, 8)
    attn_ctx.close()  # free attention PSUM/SBUF pools before out-proj

    # --- output projection -------------------------------------------------
    with (
        tc.tile_pool(name="ops", bufs=2, space="PSUM") as ops,
        tc.tile_pool(name="osb", bufs=3) as osb,
    ):
        for tt in range(NTT):
            ps = ops.tile([128, 1024], F32, tag="o")
            for c0, cn in ((0, 512), (512, 64)):
                for ki in range(NKT):
                    nc.tensor.matmul(
                        ps[:, c0 : c0 + cn],
                        lhsT=yT[ki][:, 128 * tt : 128 * (tt + 1)],
                        rhs=wo_sb[: _kw(ki), ki, c0 : c0 + cn],
                        start=(ki == 0),
                        stop=(ki == NKT - 1),
                    )
            o_sb = osb.tile([128, C], F32, tag="ot")
            nc.vector.tensor_copy(o_sb, ps[:, 0:C])
            nc.sync.dma_start(out=out[128 * tt : 128 * (tt + 1), :], in_=o_sb)


def build_bass():
    # Bacc (not raw Bass): its finalize() runs move_matmul_waits_to_ldweights
    # + generate_event_semaphores, required to satisfy the 1-wait-per-
    # instruction hardware constraint that walrus enforces.
    nc = bacc.Bacc("TRN2", target_bir_lowering=False)
    x = nc.declare_dram_parameter("x", [T, CPAD], BF16, isOutput=False)
    wq = nc.declare_dram_parameter("wq", [128, NKT, C], BF16, isOutput=False)
    wk = nc.declare_dram_parameter("wk", [128, NKT, KVC], BF16, isOutput=False)
    wv = nc.declare_dram_parameter("wv", [128, NKT, KVC], BF16, isOutput=False)
    wo = nc.declare_dram_parameter("wo", [128, NKT, C], BF16, isOutput=False)
    out = nc.declare_dram_parameter("out", [T, C], F32, isOutput=True)
    with tile.TileContext(nc) as tc, ExitStack() as ctx:
        build_kernel(tc, ctx, x[:], wq[:], wk[:], wv[:], wo[:], out[:])
    nc.finalize()  # runs Bacc.compile(): reg alloc + wait splitting
    return nc


_NC_CACHE = None


def _get_nc():
    global _NC_CACHE
    if _NC_CACHE is None:
        _NC_CACHE = build_bass()
    return _NC_CACHE


def prep_inputs(x, Wq, bq, Wk, bk, Wv, bv, Wo, bo):
    """Host-side: fold biases into an extra weight row, pad weights to 640
    rows (single strided DMA each), cast bf16, pad x with a ones column."""
    bf = ml_dtypes.bfloat16

    def wpad(W, b):
        """pad to 640 rows (bias at row 576), interleave to [128, 5, cols]
        with [p, ki, c] = row 128*ki + p (the SBUF layout, one flat DMA)"""
        cols = W.shape[1]
        wp = np.zeros((CPAD, cols), dtype=bf)
        wp[: W.shape[0]] = W.astype(bf)
        wp[W.shape[0]] = b.astype(bf)
        return np.ascontiguousarray(
            wp.reshape(NKT, 128, cols).transpose(1, 0, 2)
        )

    wq = wpad(Wq, bq)
    wk = wpad(Wk, bk)
    wv = wpad(Wv, bv)
    wo = wpad(Wo, bo)
    x = np.asarray(x, dtype=np.float32)
    xp = np.zeros((B, T, CPAD), dtype=bf)
    xp[:, :, :C] = x.astype(bf)
    xp[:, :, C] = 1.0
    in_maps = [
        {"x": np.ascontiguousarray(xp[b]), "wq": wq, "wk": wk, "wv": wv, "wo": wo}
        for b in range(N_CORES)
    ]
    return in_maps


def kernel(x, Wq, bq, Wk, bk, Wv, bv, Wo, bo, _trace=False, _trace_kwargs=None):
    nc = _get_nc()
    in_maps = prep_inputs(x, Wq, bq, Wk, bk, Wv, bv, Wo, bo)
    res = run_bass_kernel_spmd(
        nc,
        in_maps,
        core_ids=list(range(N_CORES)),
        trace=_trace,
        **(_trace_kwargs or {}),
    )
    out = np.stack([res.results[b]["out"] for b in range(N_CORES)], axis=0)
    if _trace:
        return out.astype(np.float32), res
    return out.astype(np.float32)


# revision 34
# speedup vs baseline: 1.2520x; 1.0829x over previous
"""Causal GQA multi-head attention on 8 TRN2 NeuronCores.

Sharding: data-parallel over batch (B=8 -> one batch element per core,
weights replicated, no collectives).

Per-core kernel (T=1024, C=576, 9 q-heads / 3 kv-heads, hd=64):
  - x arrives host-padded to [T, 640] bf16 (col 576 = ones column for the
    bias trick, 577.. = 0) and is transposed on-chip by the DMA xbar into
    channel-major xT tiles. Weights arrive host-padded to 640 rows so each
    loads with a single 3D-strided DMA.
  - Projections fold biases in via the ones row. q is projected in
    head-PAIR m-tiles (M=128) into qP pair tiles; k per kv-head, with
    shifted duplicates so each score pair has its lhsT at both partition
    0 and 64.
  - Scores are computed transposed, S^T[tk, tq] = k-block x qT, causal by
    construction (only lower j-blocks). The two heads of a pair run as
    CONCURRENT matmuls in disjoint PE row-groups (K=64 each). The 8
    j-blocks pack into 4 psum tiles -> 4 exp() ScalarE ops per head with
    zero junk. Diagonal masking = post-exp multiply by a 0/1 triangular
    mask on GpSimd.
  - PV accumulates yT_aug[h] = [v|1].T @ P~^T directly in transposed
    layout ([65, T] psum; row 64 = softmax denominator). DMA routes rows
    into shared yT tiles; denominators are batch-inverted with
    reciprocal_approx_fast and DMA-broadcast, then multiplied in.
  - out = yT.T @ (Wo|bo).
"""

import sys

for _p in ("/opt/trn_rl_repo",):
    if _p not in sys.path:
        sys.path.insert(0, _p)

from contextlib import ExitStack

import ml_dtypes
import numpy as np

import concourse.bass as bass
import concourse.mybir as mybir
import concourse.tile as tile
from concourse import bacc
from concourse.bass_utils import run_bass_kernel_spmd
from concourse.masks import make_upper_triangular

B, T, C = 8, 1024, 576
NH, NKV, HD = 9, 3, 64
KVC = 192
NREP = NH // NKV  # 3
NKT = 5  # channel k-tiles: 4 x 128 + 64(+1 ones row)
NTT = T // 128  # 8 token tiles
CPAD = 640  # host-padded width: C + ones col/row + zeros
F32 = mybir.dt.float32
BF16 = mybir.dt.bfloat16
SCALE = 1.0 / float(np.sqrt(HD))

N_CORES = 8

# causal j-block -> (exp tile, column offset); packs the 8 blocks of a head
# into 5 two-bank psum tiles with no junk columns.
J_MAP = {0: (0, 0), 4: (1, 0), 5: (1, 512), 1: (2, 0), 7: (2, 896),
         2: (3, 0), 6: (3, 768), 3: (4, 0)}
EXP_LEN = {0: 1024, 1: 896, 2: 1024, 3: 1024, 4: 640}
NSTEP = 5
SC_W = 1024  # score psum tile width (2 banks)

# head pairs for concurrent score matmuls: (head_even, head_odd)
PAIRS = [(0, 1), (2, 3), (4, 5), (6, 7)]
# Use concurrent PE row-group pairs for scores (False: sequential base-0)
PAIRED_SCORES = True


def _kw(ki):
    """channel-tile rows as matmul K (incl. ones row on last tile)"""
    return 128 if ki < NKT - 1 else C - 128 * (NKT - 1) + 1  # 65


def build_kernel(tc, ctx, x, wq, wk, wv, wo, out):
    nc = tc.nc

    consts = ctx.enter_context(tc.tile_pool(name="consts", bufs=1))
    persist = ctx.enter_context(tc.tile_pool(name="persist", bufs=1))
    dram_pool = ctx.enter_context(tc.tile_pool(name="dram", bufs=1, space="DRAM"))

    # --- xT via DMA xbar transpose, issued first (sync + scalar queues) ---
    # weights: host-interleaved to [128, NKT, cols] so each is ONE contiguous
    # 2D DMA; tile [p, ki, c] = original row 128*ki + p
    def load_w(w_ap, cols, tag, eng):
        t = consts.tile([128, NKT, cols], BF16, tag=tag, name=tag)
        eng.dma_start(out=t, in_=w_ap)
        return t

    wv_sb = load_w(wv, KVC, "wv", nc.sync)  # first: v-proj is first consumer
    wk_sb = load_w(wk, KVC, "wk", nc.scalar)
    xT = []
    for ki in range(NKT):
        xT_t = persist.tile([128, T], BF16, tag=f"xT{ki}", name=f"xT{ki}")
        eng = nc.sync if ki % 2 == 0 else nc.scalar
        eng.dma_start_transpose(xT_t, x[:, 128 * ki : 128 * (ki + 1)])
        xT.append(xT_t)
    wq_sb = load_w(wq, C, "wq", nc.sync)
    wo_sb = load_w(wo, C, "wo", nc.scalar)

    # --- constants ---------------------------------------------------------
    m01 = consts.tile([128, 128], BF16, tag="m01")
    make_upper_triangular(nc, m01, val=1.0, diag=True)

    # --- persistent activations -------------------------------------------
    yT = []  # channel-major attention out, ones row on last tile
    for ki in range(NKT):
        yT_t = persist.tile([_kw(ki), T], BF16, tag=f"yT{ki}", name=f"yT{ki}")
        yT.append(yT_t)
    nc.vector.memset(yT[NKT - 1][HD : HD + 1, :], 1.0)

    # q pair tiles (rows 0-63 = even head, 64-127 = odd head), q8 solo
    qP = [persist.tile([128, T], BF16, tag=f"qP{p}", name=f"qP{p}")
          for p in range(4)]
    q8 = persist.tile([HD, T], BF16, tag="q8")
    if not PAIRED_SCORES:  # odd heads shifted to base partition 0
        qOdd = [persist.tile([HD, T], BF16, tag=f"qO{p}", name=f"qO{p}")
                for p in range(4)]
    # k: kpair = [K0|K1] (projection layout), k2 = K2, plus shifted dups so
    # every pair has its lhsT at the right base partition:
    #   dup1 = [K1|K0], dup2 = [.|K2]
    kpair = persist.tile([128, T], BF16, tag="kpair")
    k2 = persist.tile([HD, T], BF16, tag="k2")
    dup1 = persist.tile([128, T], BF16, tag="dup1")
    dup2 = persist.tile([128, T], BF16, tag="dup2")

    v_aug = []  # per token tile: [128, NKV, 65]; col 64 = ones
    for tt in range(NTT):
        v_t = persist.tile([128, NKV, HD + 1], BF16, tag=f"v{tt}", name=f"v{tt}")
        nc.vector.memset(v_t[:, :, HD : HD + 1], 1.0)
        v_aug.append(v_t)

    # softmax denominator rows: heads 0-3, 4-7, 8 (separate tiles so each
    # z-chain can start as soon as its heads finish; base partition 0 each)
    l4a = persist.tile([4, T], BF16, tag="l4a")
    l4b = persist.tile([4, T], BF16, tag="l4b")
    l1 = persist.tile([1, T], BF16, tag="l1")
    # reciprocal denominators broadcast to 64 rows per head (yT layout)
    Z = [persist.tile([128 if ki < NKT - 1 else HD, T], BF16,
                      tag=f"Z{ki}", name=f"Z{ki}") for ki in range(NKT)]
    zdram = dram_pool.tile([NH, T], BF16)

    attn_ctx = ctx.enter_context(ExitStack())
    sc_ps = attn_ctx.enter_context(tc.tile_pool(name="sc", bufs=2, space="PSUM"))
    pv_ps = attn_ctx.enter_context(tc.tile_pool(name="pv", bufs=1, space="PSUM"))
    pexp = attn_ctx.enter_context(tc.tile_pool(name="pexp", bufs=1))
    ybf_pool = attn_ctx.enter_context(tc.tile_pool(name="ybf", bufs=3))
    zpool = attn_ctx.enter_context(tc.tile_pool(name="zp", bufs=1))

    # --- v projection (token-major, ones col added by memset above) -------
    for tt in range(NTT):
        ps = sc_ps.tile([128, 1536], F32, tag="s", name=f"vp{tt}")
        for ki in range(NKT):
            nc.tensor.matmul(
                ps[:, 0:KVC],
                lhsT=xT[ki][: _kw(ki), 128 * tt : 128 * (tt + 1)],
                rhs=wv_sb[: _kw(ki), ki, :],
                start=(ki == 0),
                stop=(ki == NKT - 1),
            )
        nc.vector.tensor_copy(
            v_aug[tt][:, :, 0:HD],
            ps[:, 0:KVC].rearrange("p (a d) -> p a d", a=NKV),
        )

    # --- k projection: [K0|K1] pair m-tile + K2, then shifted dups --------
    for mi, (mw, dst) in enumerate(((128, kpair), (64, k2))):
        for ni in range(2):
            n0 = 512 * ni
            ps = sc_ps.tile([128, 1536], F32, tag="s", name=f"kp{mi}_{ni}")
            for ki in range(NKT):
                nc.tensor.matmul(
                    ps[:mw, 0:512],
                    lhsT=wk_sb[: _kw(ki), ki, 128 * mi : 128 * mi + mw],
                    rhs=xT[ki][: _kw(ki), n0 : n0 + 512],
                    start=(ki == 0),
                    stop=(ki == NKT - 1),
                )
            nc.vector.tensor_copy(dst[:mw, n0 : n0 + 512], ps[:mw, 0:512])
    nc.sync.dma_start(out=dup1[0:HD, :], in_=kpair[HD:128, :])  # K1 -> base 0
    if PAIRED_SCORES:
        nc.sync.dma_start(out=dup1[HD:128, :], in_=kpair[0:HD, :])  # K0 -> b64
        nc.sync.dma_start(out=dup2[HD:128, :], in_=k2[0:HD, :])  # K2 -> b64

    # lhsT source for each pair: (A at base 0, B at base 64 when paired,
    # else base 0)
    K_A = [kpair[0:HD, :], kpair[0:HD, :], dup1[0:HD, :], k2[0:HD, :]]
    if PAIRED_SCORES:
        K_B = [dup1[HD:128, :], kpair[HD:128, :], kpair[HD:128, :],
               dup2[HD:128, :]]
    else:
        K_B = [kpair[0:HD, :], dup1[0:HD, :], dup1[0:HD, :], k2[0:HD, :]]

    # --- attention head pairs ---------------------------------------------
    def emit_scores(ps, lhsT, rhs_src, t):
        """score matmuls for exp-tile t into psum ps (512-col bank chunks)"""
        for j, (tj, off) in J_MAP.items():
            if tj != t:
                continue
            nq = T - 128 * j
            c = off
            while c < off + nq:
                ce = min((c // 512 + 1) * 512, off + nq)
                nc.tensor.matmul(
                    ps[:, c:ce],
                    lhsT=lhsT[:, 128 * j : 128 * (j + 1)],
                    rhs=rhs_src[:, 128 * j + (c - off) : 128 * j + (ce - off)],
                    start=True,
                    stop=True,
                )
                c = ce

    def emit_exp_mask(ps, hl, t, h):
        pt = pexp.tile([128, SC_W], BF16, tag=f"p{hl}_{t}", name=f"p{h}_{t}")
        nL = EXP_LEN[t]
        nc.scalar.activation(
            pt[:, 0:nL], ps[:, 0:nL], mybir.ActivationFunctionType.Exp
        )
        for j, (tj, off) in J_MAP.items():
            if tj == t:
                nc.vector.tensor_mul(
                    pt[:, off : off + 128], pt[:, off : off + 128], m01
                )
        return pt

    def emit_pv_evac(h, ptiles, l_dst):
        g = h // NREP
        yps = pv_ps.tile([HD + 1, T], F32, tag="yh", name=f"yps{h}")
        for j in range(NTT):
            t, off = J_MAP[j]
            if j < 4:
                chunks = [(128 * j, 512 - 128 * j), (512, 512)]
            else:
                chunks = [(128 * j, 1024 - 128 * j)]
            for c0, cn in chunks:
                nc.tensor.matmul(
                    yps[:, c0 : c0 + cn],
                    lhsT=v_aug[j][:, g, :],
                    rhs=ptiles[t][:, off + c0 - 128 * j : off + c0 - 128 * j + cn],
                    start=(j == 0),
                    stop=((j == 3 and c0 < 512) or j == 7),
                )
        ybf = ybf_pool.tile([HD + 1, T], BF16, tag="ybf", name=f"ybf{h}")
        nc.vector.tensor_copy(ybf, yps)
        ki, po = divmod(HD * h, 128)
        nc.sync.dma_start(out=yT[ki][po : po + HD, :], in_=ybf[0:HD, :])
        nc.sync.dma_start(out=l_dst, in_=ybf[HD : HD + 1, :])

    # head 8 first (solo, K=64 at base 0 only) -- its z-chain then overlaps
    # the pair processing
    for ni in range(2):
        n0 = 512 * ni
        ps = sc_ps.tile([128, SC_W], F32, tag="s", name=f"qp8_{ni}")
        for ki in range(NKT):
            nc.tensor.matmul(
                ps[:HD, 0:512],
                lhsT=wq_sb[: _kw(ki), ki, 512 : 512 + HD],
                rhs=xT[ki][: _kw(ki), n0 : n0 + 512],
                start=(ki == 0),
                stop=(ki == NKT - 1),
            )
        nc.vector.tensor_scalar_mul(q8[:, n0 : n0 + 512], ps[:HD, 0:512], SCALE)
    p8 = []
    for t in range(NSTEP):
        ps = sc_ps.tile([128, SC_W], F32, tag="s", name=f"s8_{t}")
        emit_scores(ps, k2[0:HD, :], q8, t)
        p8.append(emit_exp_mask(ps, 0, t, 8))
    emit_pv_evac(8, p8, l1[0:1, :])

    def pair_block(p):
        ha, hb = PAIRS[p]
        # q projection for the pair (M=128 m-tile), scaled while evacuating
        for ni in range(2):
            n0 = 512 * ni
            ps = sc_ps.tile([128, SC_W], F32, tag="s", name=f"qp{p}_{ni}")
            for ki in range(NKT):
                nc.tensor.matmul(
                    ps[:, 0:512],
                    lhsT=wq_sb[: _kw(ki), ki, 128 * p : 128 * (p + 1)],
                    rhs=xT[ki][: _kw(ki), n0 : n0 + 512],
                    start=(ki == 0),
                    stop=(ki == NKT - 1),
                )
            nc.vector.tensor_scalar_mul(qP[p][:, n0 : n0 + 512], ps[:, 0:512], SCALE)
        if PAIRED_SCORES:
            rhs_b = qP[p][HD:128, :]
        else:
            nc.sync.dma_start(out=qOdd[p], in_=qP[p][HD:128, :])
            rhs_b = qOdd[p]
        # scores for both heads (concurrent PE row groups when paired)
        pa, pb = [], []
        for t in range(NSTEP):
            psA = sc_ps.tile([128, SC_W], F32, tag="s", name=f"sA{p}_{t}")
            psB = sc_ps.tile([128, SC_W], F32, tag="s", name=f"sB{p}_{t}")
            emit_scores(psA, K_A[p], qP[p][0:HD, :], t)
            emit_scores(psB, K_B[p], rhs_b, t)
            pa.append(emit_exp_mask(psA, 0, t, ha))
            pb.append(emit_exp_mask(psB, 1, t, hb))
        lt = l4a if ha < 4 else l4b
        emit_pv_evac(ha, pa, lt[ha % 4 : ha % 4 + 1, :])
        emit_pv_evac(hb, pb, lt[hb % 4 : hb % 4 + 1, :])

    def z_chain(l_src, rows, zrow0):
        lf = zpool.tile([rows, T], F32, tag=f"lf{zrow0}", name=f"lf{zrow0}")
        nc.vector.tensor_copy(lf, l_src)
        zf = zpool.tile([rows, T], F32, tag=f"zf{zrow0}", name=f"zf{zrow0}")
        nc.vector.reciprocal_approx_fast(zf, lf)
        zb = zpool.tile([rows, T], BF16, tag=f"zb{zrow0}", name=f"zb{zrow0}")
        nc.vector.tensor_copy(zb, zf)
        nc.sync.dma_start(out=zdram[zrow0 : zrow0 + rows, :], in_=zb)
        for h in range(zrow0, zrow0 + rows):
            ki, po = divmod(HD * h, 128)
            zsl = zdram[h : h + 1, :]
            zbcast = bass.AP(
                tensor=zsl.tensor,
                offset=zsl.offset,
                ap=[[0, HD], list(zsl.ap[1])],
            )
            nc.gpsimd.dma_start(out=Z[ki][po : po + HD, :], in_=zbcast)
            nc.vector.tensor_mul(
                yT[ki][po : po + HD, :],
                yT[ki][po : po + HD, :],
                Z[ki][po : po + HD, :],
            )

    z_chain(l1, 1, 8)  # head 8's normalize overlaps the pairs below
    pair_block(0)
    pair_block(1)
    z_chain(l4a, 4, 0)  # heads 0-3 normalize overlaps pairs 2-3
    pair_block(2)
    pair_block(3)
    z_chain(l4b, 4, 4)
    attn_ctx.close()  # free attention PSUM/SBUF pools before out-proj

    # --- output projection -------------------------------------------------
    with (
        tc.tile_pool(name="ops", bufs=2, space="PSUM") as ops,
        tc.tile_pool(name="osb", bufs=3) as osb,
    ):
        for tt in range(NTT):
            ps = ops.tile([128, 1024], F32, tag="o")
            for c0, cn in ((0, 512), (512, 64)):
                for ki in range(NKT):
                    nc.tensor.matmul(
                        ps[:, c0 : c0 + cn],
                        lhsT=yT[ki][:, 128 * tt : 128 * (tt + 1)],
                        rhs=wo_sb[: _kw(ki), ki, c0 : c0 + cn],
                        start=(ki == 0),
                        stop=(ki == NKT - 1),
                    )
            o_sb = osb.tile([128, C], F32, tag="ot")
            nc.vector.tensor_copy(o_sb, ps[:, 0:C])
            nc.sync.dma_start(out=out[128 * tt : 128 * (tt + 1), :], in_=o_sb)


def build_bass():
    # Bacc (not raw Bass): its finalize() runs move_matmul_waits_to_ldweights
    # + generate_event_semaphores, required to satisfy the 1-wait-per-
    # instruction hardware constraint that walrus enforces.
    nc = bacc.Bacc("TRN2", target_bir_lowering=False)
    x = nc.declare_dram_parameter("x", [T, CPAD], BF16, isOutput=False)
    wq = nc.declare_dram_parameter("wq", [128, NKT, C], BF16, isOutput=False)
    wk = nc.declare_dram_parameter("wk", [128, NKT, KVC], BF16, isOutput=False)
    wv = nc.declare_dram_parameter("wv", [128, NKT, KVC], BF16, isOutput=False)
    wo = nc.declare_dram_parameter("wo", [128, NKT, C], BF16, isOutput=False)
    out = nc.declare_dram_parameter("out", [T, C], F32, isOutput=True)
    with tile.TileContext(nc) as tc, ExitStack() as ctx:
        build_kernel(tc, ctx, x[:], wq[:], wk[:], wv[:], wo[:], out[:])
    nc.finalize()  # runs Bacc.compile(): reg alloc + wait splitting
    return nc


_NC_CACHE = None


def _get_nc():
    global _NC_CACHE
    if _NC_CACHE is None:
        _NC_CACHE = build_bass()
    return _NC_CACHE


def prep_inputs(x, Wq, bq, Wk, bk, Wv, bv, Wo, bo):
    """Host-side: fold biases into an extra weight row, pad weights to 640
    rows (single strided DMA each), cast bf16, pad x with a ones column."""
    bf = ml_dtypes.bfloat16

    def wpad(W, b):
        """pad to 640 rows (bias at row 576), interleave to [128, 5, cols]
        with [p, ki, c] = row 128*ki + p (the SBUF layout, one flat DMA)"""
        cols = W.shape[1]
        wp = np.zeros((CPAD, cols), dtype=bf)
        wp[: W.shape[0]] = W.astype(bf)
        wp[W.shape[0]] = b.astype(bf)
        return np.ascontiguousarray(
            wp.reshape(NKT, 128, cols).transpose(1, 0, 2)
        )

    wq = wpad(Wq, bq)
    wk = wpad(Wk, bk)
    wv = wpad(Wv, bv)
    wo = wpad(Wo, bo)
    x = np.asarray(x, dtype=np.float32)
    xp = np.zeros((B, T, CPAD), dtype=bf)
    xp[:, :, :C] = x.astype(bf)
    xp[:, :, C] = 1.0
    in_maps = [
        {"x": np.ascontiguousarray(xp[b]), "wq": wq, "wk": wk, "wv": wv, "wo": wo}
        for b in range(N_CORES)
    ]
    return in_maps


def kernel(x, Wq, bq, Wk, bk, Wv, bv, Wo, bo, _trace=False, _trace_kwargs=None):
    nc = _get_nc()
    in_maps = prep_inputs(x, Wq, bq, Wk, bk, Wv, bv, Wo, bo)
    res = run_bass_kernel_spmd(
        nc,
        in_maps,
        core_ids=list(range(N_CORES)),
        trace=_trace,
        **(_trace_kwargs or {}),
    )
    out = np.stack([res.results[b]["out"] for b in range(N_CORES)], axis=0)
    if _trace:
        return out.astype(np.float32), res
    return out.astype(np.float32)


# revision 45
# speedup vs baseline: 1.4396x; 1.1499x over previous
"""Causal GQA multi-head attention on 8 TRN2 NeuronCores.

Sharding: data-parallel over batch (B=8 -> one batch element per core,
weights replicated, no collectives).

Per-core kernel (T=1024, C=576, 9 q-heads / 3 kv-heads, hd=64):
  - x arrives host-padded to [T, 640] bf16 (col 576 = ones column for the
    bias trick, 577.. = 0) and is transposed on-chip by the DMA xbar into
    channel-major xT tiles. Weights arrive host-padded to 640 rows so each
    loads with a single 3D-strided DMA.
  - Projections fold biases in via the ones row. q is projected in
    head-PAIR m-tiles (M=128) into qP pair tiles; k per kv-head, with
    shifted duplicates so each score pair has its lhsT at both partition
    0 and 64.
  - Scores are computed transposed, S^T[tk, tq] = k-block x qT, causal by
    construction (only lower j-blocks). The two heads of a pair run as
    CONCURRENT matmuls in disjoint PE row-groups (K=64 each). The 8
    j-blocks pack into 4 psum tiles -> 4 exp() ScalarE ops per head with
    zero junk. Diagonal masking = post-exp multiply by a 0/1 triangular
    mask on GpSimd.
  - PV accumulates yT_aug[h] = [v|1].T @ P~^T directly in transposed
    layout ([65, T] psum; row 64 = softmax denominator). DMA routes rows
    into shared yT tiles; denominators are batch-inverted with
    reciprocal_approx_fast and DMA-broadcast, then multiplied in.
  - out = yT.T @ (Wo|bo).
"""

import sys

for _p in ("/opt/trn_rl_repo",):
    if _p not in sys.path:
        sys.path.insert(0, _p)

from contextlib import ExitStack

import ml_dtypes
import numpy as np

import concourse.bass as bass
import concourse.mybir as mybir
import concourse.tile as tile
from concourse import bacc
from concourse.bass_utils import run_bass_kernel_spmd
from concourse.masks import make_upper_triangular

B, T, C = 8, 1024, 576
NH, NKV, HD = 9, 3, 64
KVC = 192
NREP = NH // NKV  # 3
NKT = 5  # channel k-tiles: 4 x 128 + 64(+1 ones row)
NTT = T // 128  # 8 token tiles
CPAD = 640  # host-padded width: C + ones col/row + zeros
F32 = mybir.dt.float32
BF16 = mybir.dt.bfloat16
SCALE = 1.0 / float(np.sqrt(HD))

N_CORES = 8

# causal j-block -> (exp tile, column offset); packs the 8 blocks of a head
# into 5 two-bank psum tiles with no junk columns.
J_MAP = {0: (0, 0), 4: (1, 0), 5: (1, 512), 1: (2, 0), 7: (2, 896),
         2: (3, 0), 6: (3, 768), 3: (4, 0)}
EXP_LEN = {0: 1024, 1: 896, 2: 1024, 3: 1024, 4: 640}
NSTEP = 5
SC_W = 1024  # score psum tile width (2 banks)

# head pairs for concurrent score matmuls: (head_even, head_odd)
PAIRS = [(0, 1), (2, 3), (4, 5), (6, 7)]
# Use concurrent PE row-group pairs for scores (False: sequential base-0)
PAIRED_SCORES = True


def _kw(ki):
    """channel-tile rows as matmul K (incl. ones row on last tile)"""
    return 128 if ki < NKT - 1 else C - 128 * (NKT - 1) + 1  # 65


def build_kernel(tc, ctx, x, wq, wk, wv, wo, out):
    nc = tc.nc

    consts = ctx.enter_context(tc.tile_pool(name="consts", bufs=1))
    persist = ctx.enter_context(tc.tile_pool(name="persist", bufs=1))

    # --- xT via DMA xbar transpose, issued first (sync + scalar queues) ---
    # weights: host-interleaved to [128, NKT, cols] so each is ONE contiguous
    # 2D DMA; tile [p, ki, c] = original row 128*ki + p
    def load_w(w_ap, cols, tag, eng):
        t = consts.tile([128, NKT, cols], BF16, tag=tag, name=tag)
        eng.dma_start(out=t, in_=w_ap)
        return t

    wv_sb = load_w(wv, KVC, "wv", nc.sync)  # first: v-proj is first consumer
    wk_sb = load_w(wk, KVC, "wk", nc.scalar)
    xT = []
    for ki in range(NKT):
        xT_t = persist.tile([128, T], BF16, tag=f"xT{ki}", name=f"xT{ki}")
        eng = nc.sync if ki % 2 == 0 else nc.scalar
        eng.dma_start_transpose(xT_t, x[:, 128 * ki : 128 * (ki + 1)])
        xT.append(xT_t)
    wq_sb = load_w(wq, C, "wq", nc.sync)
    wo_sb = load_w(wo, C, "wo", nc.scalar)

    # --- constants ---------------------------------------------------------
    m01 = consts.tile([128, 128], BF16, tag="m01")
    make_upper_triangular(nc, m01, val=1.0, diag=True)
    # selector for the z broadcast matmul: out row m gets z row 0 (m<64) or
    # row 1 (m>=64)
    # sel2[k, m] = 1 iff m//64 == k, built via two affine selects
    sel2 = consts.tile([2, 128], BF16, tag="sel2")
    nc.gpsimd.memset(sel2, 1.0)
    nc.gpsimd.affine_select(  # keep where 64*x - y + 63 >= 0
        out=sel2, in_=sel2, compare_op=mybir.AluOpType.is_ge, fill=0.0,
        base=63, pattern=[[-1, 128]], channel_multiplier=64,
    )
    nc.gpsimd.affine_select(  # keep where y - 64*x >= 0
        out=sel2, in_=sel2, compare_op=mybir.AluOpType.is_ge, fill=0.0,
        base=0, pattern=[[1, 128]], channel_multiplier=-64,
    )

    # --- persistent activations -------------------------------------------
    yT = []  # channel-major attention out, ones row on last tile
    for ki in range(NKT):
        yT_t = persist.tile([_kw(ki), T], BF16, tag=f"yT{ki}", name=f"yT{ki}")
        yT.append(yT_t)
    nc.vector.memset(yT[NKT - 1][HD : HD + 1, :], 1.0)

    # q pair tiles (rows 0-63 = even head, 64-127 = odd head), q8 solo
    qP = [persist.tile([128, T], BF16, tag=f"qP{p}", name=f"qP{p}")
          for p in range(4)]
    q8 = persist.tile([HD, T], BF16, tag="q8")
    if not PAIRED_SCORES:  # odd heads shifted to base partition 0
        qOdd = [persist.tile([HD, T], BF16, tag=f"qO{p}", name=f"qO{p}")
                for p in range(4)]
    # k: kpair = [K0|K1] (projection layout), k2 = K2, plus shifted dups so
    # every pair has its lhsT at the right base partition:
    #   dup1 = [K1|K0], dup2 = [.|K2]
    kpair = persist.tile([128, T], BF16, tag="kpair")
    k2 = persist.tile([HD, T], BF16, tag="k2")
    dup1 = persist.tile([128, T], BF16, tag="dup1")
    dup2 = persist.tile([128, T], BF16, tag="dup2")

    v_aug = []  # per token tile: [128, NKV, 65]; col 64 = ones
    for tt in range(NTT):
        v_t = persist.tile([128, NKV, HD + 1], BF16, tag=f"v{tt}", name=f"v{tt}")
        nc.vector.memset(v_t[:, :, HD : HD + 1], 1.0)
        v_aug.append(v_t)

    # softmax denominator rows, per pair (base partition 0 each)
    l2 = [persist.tile([2, T], BF16, tag=f"l2_{p}", name=f"l2_{p}")
          for p in range(4)]
    l1 = persist.tile([1, T], BF16, tag="l1")

    attn_ctx = ctx.enter_context(ExitStack())
    sc_ps = attn_ctx.enter_context(tc.tile_pool(name="sc", bufs=3, space="PSUM"))
    pv_ps = attn_ctx.enter_context(tc.tile_pool(name="pv", bufs=1, space="PSUM"))
    pexp = attn_ctx.enter_context(tc.tile_pool(name="pexp", bufs=2))
    ybf_pool = attn_ctx.enter_context(tc.tile_pool(name="ybf", bufs=3))
    zpool = attn_ctx.enter_context(tc.tile_pool(name="zp", bufs=2))

    # --- v projection (token-major, ones col added by memset above) -------
    for tt in range(NTT):
        ps = sc_ps.tile([128, SC_W], F32, tag="s", name=f"vp{tt}")
        for ki in range(NKT):
            nc.tensor.matmul(
                ps[:, 0:KVC],
                lhsT=xT[ki][: _kw(ki), 128 * tt : 128 * (tt + 1)],
                rhs=wv_sb[: _kw(ki), ki, :],
                start=(ki == 0),
                stop=(ki == NKT - 1),
            )
        nc.vector.tensor_copy(
            v_aug[tt][:, :, 0:HD],
            ps[:, 0:KVC].rearrange("p (a d) -> p a d", a=NKV),
        )

    # --- k projection: [K0|K1] pair m-tile + K2, then shifted dups --------
    for mi, (mw, dst) in enumerate(((128, kpair), (64, k2))):
        for ni in range(2):
            n0 = 512 * ni
            ps = sc_ps.tile([128, SC_W], F32, tag="s", name=f"kp{mi}_{ni}")
            for ki in range(NKT):
                nc.tensor.matmul(
                    ps[:mw, 0:512],
                    lhsT=wk_sb[: _kw(ki), ki, 128 * mi : 128 * mi + mw],
                    rhs=xT[ki][: _kw(ki), n0 : n0 + 512],
                    start=(ki == 0),
                    stop=(ki == NKT - 1),
                )
            nc.vector.tensor_copy(dst[:mw, n0 : n0 + 512], ps[:mw, 0:512])
    nc.sync.dma_start(out=dup1[0:HD, :], in_=kpair[HD:128, :])  # K1 -> base 0
    if PAIRED_SCORES:
        nc.sync.dma_start(out=dup1[HD:128, :], in_=kpair[0:HD, :])  # K0 -> b64
        nc.sync.dma_start(out=dup2[HD:128, :], in_=k2[0:HD, :])  # K2 -> b64

    # lhsT source for each pair: (A at base 0, B at base 64 when paired,
    # else base 0)
    K_A = [kpair[0:HD, :], kpair[0:HD, :], dup1[0:HD, :], k2[0:HD, :]]
    if PAIRED_SCORES:
        K_B = [dup1[HD:128, :], kpair[HD:128, :], kpair[HD:128, :],
               dup2[HD:128, :]]
    else:
        K_B = [kpair[0:HD, :], dup1[0:HD, :], dup1[0:HD, :], k2[0:HD, :]]

    # --- attention head pairs ---------------------------------------------
    def emit_scores(ps, lhsT, rhs_src, t):
        """score matmuls for exp-tile t into psum ps (512-col bank chunks)"""
        for j, (tj, off) in J_MAP.items():
            if tj != t:
                continue
            nq = T - 128 * j
            c = off
            while c < off + nq:
                ce = min((c // 512 + 1) * 512, off + nq)
                nc.tensor.matmul(
                    ps[:, c:ce],
                    lhsT=lhsT[:, 128 * j : 128 * (j + 1)],
                    rhs=rhs_src[:, 128 * j + (c - off) : 128 * j + (ce - off)],
                    start=True,
                    stop=True,
                )
                c = ce

    def emit_exp_mask(ps, hl, t, h):
        pt = pexp.tile([128, SC_W], BF16, tag=f"p{hl}_{t}", name=f"p{h}_{t}")
        nL = EXP_LEN[t]
        nc.scalar.activation(
            pt[:, 0:nL], ps[:, 0:nL], mybir.ActivationFunctionType.Exp
        )
        # diag-block masks: split across DVE (even heads) / GpSimd (odd) to
        # balance load
        eng = nc.vector if hl == 0 else nc.gpsimd
        for j, (tj, off) in J_MAP.items():
            if tj == t:
                eng.tensor_mul(
                    pt[:, off : off + 128], pt[:, off : off + 128], m01
                )
        return pt

    def emit_pv_evac(h, ptiles, l_dst):
        g = h // NREP
        yps = pv_ps.tile([HD + 1, T], F32, tag="yh", name=f"yps{h}")
        for j in range(NTT):
            t, off = J_MAP[j]
            if j < 4:
                chunks = [(128 * j, 512 - 128 * j), (512, 512)]
            else:
                chunks = [(128 * j, 1024 - 128 * j)]
            for c0, cn in chunks:
                nc.tensor.matmul(
                    yps[:, c0 : c0 + cn],
                    lhsT=v_aug[j][:, g, :],
                    rhs=ptiles[t][:, off + c0 - 128 * j : off + c0 - 128 * j + cn],
                    start=(j == 0),
                    stop=((j == 3 and c0 < 512) or j == 7),
                )
        ybf = ybf_pool.tile([HD + 1, T], BF16, tag="ybf", name=f"ybf{h}")
        nc.vector.tensor_copy(ybf, yps)
        ki, po = divmod(HD * h, 128)
        nc.sync.dma_start(out=yT[ki][po : po + HD, :], in_=ybf[0:HD, :])
        nc.sync.dma_start(out=l_dst, in_=ybf[HD : HD + 1, :])

    # head 8 first (solo, K=64 at base 0 only) -- its z-chain then overlaps
    # the pair processing
    for ni in range(2):
        n0 = 512 * ni
        ps = sc_ps.tile([128, SC_W], F32, tag="s", name=f"qp8_{ni}")
        for ki in range(NKT):
            nc.tensor.matmul(
                ps[:HD, 0:512],
                lhsT=wq_sb[: _kw(ki), ki, 512 : 512 + HD],
                rhs=xT[ki][: _kw(ki), n0 : n0 + 512],
                start=(ki == 0),
                stop=(ki == NKT - 1),
            )
        nc.vector.tensor_scalar_mul(q8[:, n0 : n0 + 512], ps[:HD, 0:512], SCALE)
    p8 = []
    for t in range(NSTEP):
        ps = sc_ps.tile([128, SC_W], F32, tag="s", name=f"s8_{t}")
        emit_scores(ps, k2[0:HD, :], q8, t)
        p8.append(emit_exp_mask(ps, 0, t, 8))
    emit_pv_evac(8, p8, l1[0:1, :])

    def emit_qproj(p):
        # q projection for the pair (M=128 m-tile), scaled while evacuating
        for ni in range(2):
            n0 = 512 * ni
            ps = sc_ps.tile([128, SC_W], F32, tag="s", name=f"qp{p}_{ni}")
            for ki in range(NKT):
                nc.tensor.matmul(
                    ps[:, 0:512],
                    lhsT=wq_sb[: _kw(ki), ki, 128 * p : 128 * (p + 1)],
                    rhs=xT[ki][: _kw(ki), n0 : n0 + 512],
                    start=(ki == 0),
                    stop=(ki == NKT - 1),
                )
            nc.vector.tensor_scalar_mul(qP[p][:, n0 : n0 + 512], ps[:, 0:512], SCALE)
        if not PAIRED_SCORES:
            nc.sync.dma_start(out=qOdd[p], in_=qP[p][HD:128, :])

    def pair_scores(p):
        ha, hb = PAIRS[p]
        rhs_b = qP[p][HD:128, :] if PAIRED_SCORES else qOdd[p]
        # scores for both heads (concurrent PE row groups when paired)
        pa, pb = [], []
        for t in range(NSTEP):
            psA = sc_ps.tile([128, SC_W], F32, tag="s", name=f"sA{p}_{t}")
            psB = sc_ps.tile([128, SC_W], F32, tag="s", name=f"sB{p}_{t}")
            emit_scores(psA, K_A[p], qP[p][0:HD, :], t)
            emit_scores(psB, K_B[p], rhs_b, t)
            pa.append(emit_exp_mask(psA, 0, t, ha))
            pb.append(emit_exp_mask(psB, 1, t, hb))
        return pa, pb

    def pair_pv(p, pa, pb):
        ha, hb = PAIRS[p]
        emit_pv_evac(ha, pa, l2[p][0:1, :])
        emit_pv_evac(hb, pb, l2[p][1:2, :])

    def z_pair(p):
        """z = 1/l for both heads of pair p; broadcast to 128 rows with a
        K=2 selector matmul; multiply into yT[p] (= exactly the pair's
        channel rows)."""
        lf = zpool.tile([2, T], F32, tag="lf", name=f"lf{p}")
        nc.vector.tensor_copy(lf, l2[p])
        zf = zpool.tile([2, T], F32, tag="zf", name=f"zf{p}")
        nc.vector.reciprocal_approx_fast(zf, lf)
        zb = zpool.tile([2, T], BF16, tag="zb", name=f"zb{p}")
        nc.vector.tensor_copy(zb, zf)
        zps = sc_ps.tile([128, SC_W], F32, tag="s", name=f"zps{p}")
        for n0 in (0, 512):
            nc.tensor.matmul(
                zps[:, n0 : n0 + 512],
                lhsT=sel2,
                rhs=zb[:, n0 : n0 + 512],
                start=True,
                stop=True,
            )
        nc.vector.tensor_mul(yT[p], yT[p], zps[:, 0:T])

    def z_h8():
        lf8 = zpool.tile([1, T], F32, tag="lf")
        nc.vector.tensor_copy(lf8, l1)
        zf8 = zpool.tile([1, T], F32, tag="zf")
        nc.vector.reciprocal_approx_fast(zf8, lf8)
        zb8 = zpool.tile([1, T], BF16, tag="zb")
        nc.vector.tensor_copy(zb8, zf8)
        zps = sc_ps.tile([128, SC_W], F32, tag="s", name="zps8")
        for n0 in (0, 512):
            nc.tensor.matmul(
                zps[:HD, n0 : n0 + 512],
                lhsT=sel2[0:1, 0:HD],
                rhs=zb8[:, n0 : n0 + 512],
                start=True,
                stop=True,
            )
        nc.vector.tensor_mul(
            yT[4][0:HD, :], yT[4][0:HD, :], zps[0:HD, 0:T]
        )

    # software-pipelined emission: next pair's q-projection lands between a
    # pair's scores and its PV so the PE never head-blocks on the exp tail
    emit_qproj(0)
    sc0 = pair_scores(0)
    emit_qproj(1)
    pair_pv(0, *sc0)
    z_h8()
    sc1 = pair_scores(1)
    emit_qproj(2)
    pair_pv(1, *sc1)
    z_pair(0)
    sc2 = pair_scores(2)
    emit_qproj(3)
    pair_pv(2, *sc2)
    z_pair(1)
    sc3 = pair_scores(3)
    pair_pv(3, *sc3)
    z_pair(2)
    z_pair(3)
    attn_ctx.close()  # free attention PSUM/SBUF pools before out-proj

    # --- output projection -------------------------------------------------
    with (
        tc.tile_pool(name="ops", bufs=2, space="PSUM") as ops,
        tc.tile_pool(name="osb", bufs=3) as osb,
    ):
        for tt in range(NTT):
            ps = ops.tile([128, 1024], F32, tag="o")
            for c0, cn in ((0, 512), (512, 64)):
                for ki in range(NKT):
                    nc.tensor.matmul(
                        ps[:, c0 : c0 + cn],
                        lhsT=yT[ki][:, 128 * tt : 128 * (tt + 1)],
                        rhs=wo_sb[: _kw(ki), ki, c0 : c0 + cn],
                        start=(ki == 0),
                        stop=(ki == NKT - 1),
                    )
            o_sb = osb.tile([128, C], F32, tag="ot")
            nc.vector.tensor_copy(o_sb, ps[:, 0:C])
            nc.sync.dma_start(out=out[128 * tt : 128 * (tt + 1), :], in_=o_sb)


def build_bass():
    # Bacc (not raw Bass): its finalize() runs move_matmul_waits_to_ldweights
    # + generate_event_semaphores, required to satisfy the 1-wait-per-
    # instruction hardware constraint that walrus enforces.
    nc = bacc.Bacc("TRN2", target_bir_lowering=False)
    x = nc.declare_dram_parameter("x", [T, CPAD], BF16, isOutput=False)
    wq = nc.declare_dram_parameter("wq", [128, NKT, C], BF16, isOutput=False)
    wk = nc.declare_dram_parameter("wk", [128, NKT, KVC], BF16, isOutput=False)
    wv = nc.declare_dram_parameter("wv", [128, NKT, KVC], BF16, isOutput=False)
    wo = nc.declare_dram_parameter("wo", [128, NKT, C], BF16, isOutput=False)
    out = nc.declare_dram_parameter("out", [T, C], F32, isOutput=True)
    with tile.TileContext(nc) as tc, ExitStack() as ctx:
        build_kernel(tc, ctx, x[:], wq[:], wk[:], wv[:], wo[:], out[:])
    nc.finalize()  # runs Bacc.compile(): reg alloc + wait splitting
    return nc


_NC_CACHE = None


def _get_nc():
    global _NC_CACHE
    if _NC_CACHE is None:
        _NC_CACHE = build_bass()
    return _NC_CACHE


def prep_inputs(x, Wq, bq, Wk, bk, Wv, bv, Wo, bo):
    """Host-side: fold biases into an extra weight row, pad weights to 640
    rows (single strided DMA each), cast bf16, pad x with a ones column."""
    bf = ml_dtypes.bfloat16

    def wpad(W, b):
        """pad to 640 rows (bias at row 576), interleave to [128, 5, cols]
        with [p, ki, c] = row 128*ki + p (the SBUF layout, one flat DMA)"""
        cols = W.shape[1]
        wp = np.zeros((CPAD, cols), dtype=bf)
        wp[: W.shape[0]] = W.astype(bf)
        wp[W.shape[0]] = b.astype(bf)
        return np.ascontiguousarray(
            wp.reshape(NKT, 128, cols).transpose(1, 0, 2)
        )

    wq = wpad(Wq, bq)
    wk = wpad(Wk, bk)
    wv = wpad(Wv, bv)
    wo = wpad(Wo, bo)
    x = np.asarray(x, dtype=np.float32)
    xp = np.zeros((B, T, CPAD), dtype=bf)
    xp[:, :, :C] = x.astype(bf)
    xp[:, :, C] = 1.0
    in_maps = [
        {"x": np.ascontiguousarray(xp[b]), "wq": wq, "wk": wk, "wv": wv, "wo": wo}
        for b in range(N_CORES)
    ]
    return in_maps


def kernel(x, Wq, bq, Wk, bk, Wv, bv, Wo, bo, _trace=False, _trace_kwargs=None):
    nc = _get_nc()
    in_maps = prep_inputs(x, Wq, bq, Wk, bk, Wv, bv, Wo, bo)
    res = run_bass_kernel_spmd(
        nc,
        in_maps,
        core_ids=list(range(N_CORES)),
        trace=_trace,
        **(_trace_kwargs or {}),
    )
    out = np.stack([res.results[b]["out"] for b in range(N_CORES)], axis=0)
    if _trace:
        return out.astype(np.float32), res
    return out.astype(np.float32)
